# revision 28
# baseline (speedup 1.0000x reference)
"""Trainium2 Bass kernel for an 8-layer dense transformer (B=4,T=1024,C=1024,
H=16,HS=64,V=32000), sharded over 8 NeuronCores as DP=4 (batch) x TP=2
(heads/FFN-hidden/vocab), with pairwise AllReduce after the attention
projection and FFN down-projection.

v2: bf16 matmul path (f32 residual/LN stats/softmax accum), host-precomputed
PhW@Pm, per-layer resident weights (single DMA per tensor per layer),
token-half pipelined layer schedule so every AllReduce overlaps independent
matmul work, batched fast reciprocals, vocab-stationary head.
"""
import numpy as np
import ml_dtypes

import concourse.bass as bass
import concourse.mybir as mybir
import concourse.tile as tile
from concourse import bacc
from concourse.masks import make_identity

V, B, T, C, H, L = 32000, 4, 1024, 1024, 16, 8
HS = C // H          # 64
P = 128
NHC = H // 2         # heads per core = 8
NPR = NHC // 2       # head pairs = 4
CK = C // P          # channel chunks = 8
F = 4 * C
FS = F // 2          # ffn shard = 2048
FCK = FS // P        # f chunks = 16
VS = V // 2          # vocab shard = 16000
VC = VS // P         # vocab chunks = 125
TH = 512             # token half
SCALE = float(C) ** -0.5
EPS = 1e-5

F32 = mybir.dt.float32
F32R = mybir.dt.float32r
BF16 = mybir.dt.bfloat16
I32 = mybir.dt.int32
AF = mybir.ActivationFunctionType
ALU = mybir.AluOpType
GROUPS = [[0, 1], [2, 3], [4, 5], [6, 7]]
BF = ml_dtypes.bfloat16

_CACHE = {}


# ----------------------------------------------------------------------------
# host-side shard prep
# ----------------------------------------------------------------------------

def _prep_core(inp, core):
    b, tp = core // 2, core % 2
    heads = slice(tp * NHC, (tp + 1) * NHC)
    f32 = lambda a: np.ascontiguousarray(np.asarray(a, np.float32))
    bf = lambda a: np.ascontiguousarray(np.asarray(a, np.float32).astype(BF))
    d = {}
    d["idx"] = np.ascontiguousarray(np.asarray(inp["idx"][b])
                                    .astype(np.int32).reshape(T, 1))
    d["tok_emb"] = bf(inp["tok_emb"])
    d["pos_emb"] = bf(inp["pos_emb"])
    for nm in ("Wq", "Wk", "Wv"):
        w = np.asarray(inp[nm], np.float32)[:, heads]          # [L,8,C,HS]
        w = np.transpose(w, (0, 2, 1, 3)).reshape(L, CK, P, NHC * HS)
        d[nm.lower()] = bf(w)
    # phpm[l] = PhW[l,heads] @ Pm[l]  -> [NPR, P, C]
    phw = np.asarray(inp["Ph_w"], np.float32)[:, heads].reshape(L, NHC * HS, C)
    pm = np.asarray(inp["Pm_w"], np.float32)
    phpm = np.einsum("lsc,lcd->lsd", phw, pm).reshape(L, NPR, P, C)
    d["phpm"] = bf(phpm)
    # w1 regrouped [L, 4(ftg), CK, P, 512]
    w1 = np.asarray(inp["W1"], np.float32)[:, :, tp * FS:(tp + 1) * FS]
    w1 = w1.reshape(L, CK, P, 4, FS // 4).transpose(0, 3, 1, 2, 4)
    d["w1"] = bf(w1)
    d["w2"] = bf(np.asarray(inp["W2"], np.float32)[:, tp * FS:(tp + 1) * FS]
                 .reshape(L, FCK, P, C))
    d["b1s"] = f32(np.asarray(inp["b1"], np.float32)[:, tp * FS:(tp + 1) * FS]
                   .reshape(L, FCK, P, 1))
    phb = np.asarray(inp["Ph_b"], np.float32)[:, heads].sum(1)   # [L, C]
    ab = np.einsum("lc,lcd->ld", phb, pm)
    if tp == 0:
        ab = ab + np.asarray(inp["Pm_b"], np.float32)
    d["ab"] = f32(ab.reshape(L, CK, P, 1))
    b2 = (np.asarray(inp["b2"], np.float32) if tp == 0
          else np.zeros((L, C), np.float32))
    d["b2g"] = f32(b2.reshape(L, CK, P, 1))
    for nm, key in (("ln1_g", "ln1g"), ("ln1_b", "ln1b"),
                    ("ln2_g", "ln2g"), ("ln2_b", "ln2b")):
        d[key] = f32(np.asarray(inp[nm]).reshape(L, CK, P, 1))
    d["lnfg"] = f32(np.asarray(inp["lnf_g"]).reshape(CK, P, 1))
    d["lnfb"] = f32(np.asarray(inp["lnf_b"]).reshape(CK, P, 1))
    d["headw"] = bf(np.asarray(inp["head_w"], np.float32)
                    [:, tp * VS:(tp + 1) * VS].reshape(CK, P, VS))
    d["headb"] = f32(np.asarray(inp["head_b"], np.float32)
                     [tp * VS:(tp + 1) * VS].reshape(VC, P, 1))
    return d


# ----------------------------------------------------------------------------
# device program
# ----------------------------------------------------------------------------

def build_nc(taps=False):
    nc = bacc.Bacc("TRN2", target_bir_lowering=False, debug=False, num_devices=8)

    def din(name, shape, dt_):
        return nc.dram_tensor(name, list(shape), dt_, kind="ExternalInput")

    idx_d = din("idx", [T, 1], I32)
    tok_d = din("tok_emb", [V, C], BF16)
    pos_d = din("pos_emb", [T, C], BF16)
    wq_d = din("wq", [L, CK, P, NHC * HS], BF16)
    wk_d = din("wk", [L, CK, P, NHC * HS], BF16)
    wv_d = din("wv", [L, CK, P, NHC * HS], BF16)
    phpm_d = din("phpm", [L, NPR, P, C], BF16)
    w1_d = din("w1", [L, 4, CK, P, FS // 4], BF16)
    w2_d = din("w2", [L, FCK, P, C], BF16)
    b1s_d = din("b1s", [L, FCK, P, 1], F32)
    ab_d = din("ab", [L, CK, P, 1], F32)
    b2g_d = din("b2g", [L, CK, P, 1], F32)
    ln1g_d = din("ln1g", [L, CK, P, 1], F32)
    ln1b_d = din("ln1b", [L, CK, P, 1], F32)
    ln2g_d = din("ln2g", [L, CK, P, 1], F32)
    ln2b_d = din("ln2b", [L, CK, P, 1], F32)
    lnfg_d = din("lnfg", [CK, P, 1], F32)
    lnfb_d = din("lnfb", [CK, P, 1], F32)
    hw_d = din("headw", [CK, P, VS], BF16)
    hb_d = din("headb", [VC, P, 1], F32)

    logits_d = nc.dram_tensor("logits", [VC, P, T], BF16, kind="ExternalOutput")

    tap_d = {}

    def mktap(nm, dt_):
        tap_d[nm] = nc.dram_tensor(nm, [P, 16], dt_, kind="ExternalOutput")

    if taps:
        for nm in ["t_emb", "t_x1", "t_x2"] + [f"t_xl{i}" for i in range(L)]:
            mktap(nm, F32)
        for nm in ["t_xn1", "t_q0", "t_k0", "t_v0", "t_ew", "t_ot", "t_sa",
                   "t_xn2", "t_h", "t_fo", "t_xf", "t_lg", "t_rr", "t_rbs",
                   "t_otu"]:
            mktap(nm, BF16)

    from contextlib import ExitStack
    with tile.TileContext(nc) as tc, ExitStack() as st:
        # ---------- persistent pools ----------
        cst = st.enter_context(tc.tile_pool(name="cst", bufs=1))
        xtp = st.enter_context(tc.tile_pool(name="xtp", bufs=1))
        big = st.enter_context(tc.tile_pool(name="big", bufs=1))
        qkt = st.enter_context(tc.tile_pool(name="qkt", bufs=1))
        vat = st.enter_context(tc.tile_pool(name="vat", bufs=1))
        ott = st.enter_context(tc.tile_pool(name="ott", bufs=1))
        wqk = st.enter_context(tc.tile_pool(name="wqk", bufs=1))
        php = st.enter_context(tc.tile_pool(name="php", bufs=1))
        w1p = st.enter_context(tc.tile_pool(name="w1p", bufs=2))
        w2p = st.enter_context(tc.tile_pool(name="w2p", bufs=4))
        ewp = st.enter_context(tc.tile_pool(name="ewp", bufs=1))
        nrm = st.enter_context(tc.tile_pool(name="nrm", bufs=2))
        sap = st.enter_context(tc.tile_pool(name="sap", bufs=1))
        arp = st.enter_context(tc.tile_pool(name="arp", bufs=2))
        xn2p = st.enter_context(tc.tile_pool(name="xn2p", bufs=1))
        drp = st.enter_context(tc.tile_pool(name="drp", bufs=8, space="DRAM"))

        xT = xtp.tile([P, CK, T], F32R)       # residual stream [c_p, c_k, t]

        cc_pending = []

        def run_cc(src_tile, th):
            bin_ = drp.tile([P, CK, TH], BF16, name="cc_in")
            bout = drp.tile([P, CK, TH], BF16, name="cc_out")
            nc.sync.dma_start(bin_[:], src_tile[:])
            nc.gpsimd.collective_compute(
                "AllReduce", ALU.add, replica_groups=GROUPS,
                ins=[bin_.opt()], outs=[bout.opt()])
            return (bout, th)

        def tap(nm, src_ap):
            if taps:
                nc.sync.dma_start(tap_d[nm].ap(), src_ap)

        def consume_cc(ccp):
            bout, th = ccp
            ts = slice(th * TH, (th + 1) * TH)
            for ct in range(CK):
                ar_c = arp.tile([P, TH], BF16, name="ar_c")
                nc.sync.dma_start(ar_c[:], bout[:, ct])
                nc.vector.tensor_add(xT[:, ct, ts], xT[:, ct, ts], ar_c[:])

        ones_f = cst.tile([P, P], F32)
        nc.vector.memset(ones_f[:], 1.0)
        ones_r = cst.tile([P, P], F32R)
        nc.vector.tensor_copy(ones_r[:], ones_f[:])
        ones_b = cst.tile([P, P], BF16)
        nc.vector.tensor_copy(ones_b[:], ones_f[:])
        ident = cst.tile([P, P], F32)
        make_identity(nc, ident[:])
        eps_b = cst.tile([P, 1], F32)
        nc.vector.memset(eps_b[:], EPS)
        m0 = cst.tile([P, P], F32)            # keep t(free) >= u(part)
        nc.gpsimd.memset(m0[:], 0.0)
        nc.gpsimd.affine_select(
            out=m0[:], in_=m0[:], compare_op=ALU.is_ge,
            fill=-1e9, base=0, pattern=[[1, P]], channel_multiplier=-1)

        def ldvec(dram, n, pat, name):
            t = cst.tile([P, n], F32, name=name)
            nc.sync.dma_start(t[:], dram.ap().rearrange(pat))
            return t

        pat3 = "l k p o -> p (l k o)"
        pat2 = "k p o -> p (k o)"
        ln1g = ldvec(ln1g_d, L * CK, pat3, "ln1g")
        ln1b = ldvec(ln1b_d, L * CK, pat3, "ln1b")
        ln2g = ldvec(ln2g_d, L * CK, pat3, "ln2g")
        ln2b = ldvec(ln2b_d, L * CK, pat3, "ln2b")
        ab_v = ldvec(ab_d, L * CK, pat3, "ab_v")
        b2_v = ldvec(b2g_d, L * CK, pat3, "b2_v")
        b1_v = ldvec(b1s_d, L * FCK, pat3, "b1_v")
        lnfg = ldvec(lnfg_d, CK, pat2, "lnfg")
        lnfb = ldvec(lnfb_d, CK, pat2, "lnfb")
        hb_v = ldvec(hb_d, VC, pat2, "hb_v")

        # ---------- embedding: gather + pos, transpose into xT ----------
        with (tc.tile_pool(name="emb", bufs=2) as emb,
              tc.tile_pool(name="embp", bufs=4, space="PSUM") as embp):
            idx_sb = emb.tile([P, CK], I32, name="idx_sb", bufs=1)
            nc.sync.dma_start(idx_sb[:],
                              idx_d.ap().rearrange("(g p) o -> p (g o)", p=P))
            for g in range(T // P):
                ge = emb.tile([P, C], BF16, name="ge")
                nc.gpsimd.indirect_dma_start(
                    out=ge[:], out_offset=None, in_=tok_d.ap(),
                    in_offset=bass.IndirectOffsetOnAxis(ap=idx_sb[:, g:g + 1],
                                                        axis=0))
                pe = emb.tile([P, C], BF16, name="pe")
                nc.sync.dma_start(pe[:], pos_d.ap()[g * P:(g + 1) * P, :])
                ge2 = emb.tile([P, C], BF16, name="ge2", bufs=1)
                nc.vector.tensor_add(ge2[:], ge[:], pe[:])
                for k in range(CK):
                    tt = emb.tile([P, P], BF16, name="tt", bufs=4)
                    nc.sync.dma_start_transpose(tt[:],
                                                ge2[:, k * P:(k + 1) * P])
                    nc.scalar.activation(xT[:, k, g * P:(g + 1) * P], tt[:],
                                         AF.Copy)

        tap("t_emb", xT[:, 0, :16].bitcast(F32))

        # ---------- layernorm of one token-half ----------
        def ln_half(dst, dst_ts, src, gv, bv, goff, th, sbp):
            ts = slice(th * TH, (th + 1) * TH)
            with tc.tile_pool(name="lnps", bufs=1, space="PSUM") as psp:
                sx = psp.tile([1, TH], F32, name="sx")[:]
                sq = psp.tile([1, TH], F32, name="sq")[:]
                for k in range(CK):
                    sqk = sbp.tile([P, TH], F32R, name="sqk", bufs=1)
                    nc.vector.tensor_mul(sqk[:], src[:, k, ts], src[:, k, ts])
                    nc.tensor.matmul(sx, ones_r[:, :1], src[:, k, ts],
                                     start=(k == 0), stop=(k == CK - 1))
                    nc.tensor.matmul(sq, ones_r[:, :1], sqk[:],
                                     start=(k == 0), stop=(k == CK - 1))
                mean = sbp.tile([1, TH], F32R, name="mean", bufs=1)
                nc.vector.tensor_scalar_mul(mean[:], sx, 1.0 / C)
                s1 = sbp.tile([1, TH], F32, name="s1", bufs=1)
                nc.vector.tensor_scalar_mul(s1[:], sq, 1.0 / C)
                s2 = sbp.tile([1, TH], F32, name="s2", bufs=1)
                nc.vector.tensor_mul(s2[:], mean[:].bitcast(F32),
                                     mean[:].bitcast(F32))
                nc.vector.tensor_sub(s2[:], s1[:], s2[:])
                nc.scalar.activation(s1[:], s2[:], AF.Sqrt,
                                     bias=eps_b[0:1, :1])
                nc.vector.reciprocal_approx_fast(out=s1[:], in_=s1[:])
                rr = sbp.tile([1, TH], F32R, name="rr", bufs=1)
                nc.vector.tensor_copy(rr[:], s1[:])
                bcast = psp.tile([P, 2, TH], F32, name="bcast")
                nc.tensor.matmul(bcast[:, 0, :], ones_r[:1, :], mean[:],
                                 start=True, stop=True)
                nc.tensor.matmul(bcast[:, 1, :], ones_r[:1, :], rr[:],
                                 start=True, stop=True)
                bmean_s = sbp.tile([P, TH], F32R, name="bmean_s", bufs=1)
                nc.scalar.activation(bmean_s[:], bcast[:, 0, :], AF.Copy)
                brstd_s = sbp.tile([P, TH], F32R, name="brstd_s", bufs=1)
                nc.scalar.activation(brstd_s[:], bcast[:, 1, :], AF.Copy)
                for k in range(CK):
                    t1 = sbp.tile([P, TH], F32R, name="sqk", bufs=1)
                    nc.vector.tensor_sub(t1[:], src[:, k, ts], bmean_s[:])
                    nc.vector.tensor_mul(t1[:], t1[:], brstd_s[:])
                    nc.vector.tensor_scalar(dst[:, k, dst_ts], t1[:],
                                            gv[:, goff + k:goff + k + 1],
                                            bv[:, goff + k:goff + k + 1],
                                            ALU.mult, ALU.add)

        # ======================= layers =======================
        for l in range(L):
            # per-layer weights (single DMA each; pool reuse gives prefetch)
            wq_sb = wqk.tile([P, CK, NHC * HS], BF16, name="wq_sb")
            nc.sync.dma_start(wq_sb[:],
                              wq_d.ap()[l].rearrange("k p f -> p k f"))
            wk_sb = wqk.tile([P, CK, NHC * HS], BF16, name="wk_sb")
            nc.sync.dma_start(wk_sb[:],
                              wk_d.ap()[l].rearrange("k p f -> p k f"))
            wv_sb = wqk.tile([P, CK, NHC * HS], BF16, name="wv_sb")
            nc.sync.dma_start(wv_sb[:],
                              wv_d.ap()[l].rearrange("k p f -> p k f"))
            php_sb = php.tile([P, NPR, C], BF16, name="php_sb")
            nc.sync.dma_start(php_sb[:],
                              phpm_d.ap()[l].rearrange("o p c -> p o c"))

            xnT = big.tile([P, CK, T], BF16, name="xnT")
            qT = qkt.tile([P, NPR, T], BF16, name="qT")
            kT = qkt.tile([P, NPR, T], BF16, name="kT")
            v_aug = vat.tile([P, T // P, NHC, HS + 2], BF16, name="v_aug")
            OT = ott.tile([P, NPR, T], BF16, name="OT")

            with tc.tile_pool(name=f"lsb_{l}", bufs=2) as lsb:

                def qkv_half(th):
                    ts = slice(th * TH, (th + 1) * TH)
                    with tc.tile_pool(name="qkps", bufs=2,
                                      space="PSUM") as qkps:
                        for pr in range(NPR):
                            q_ps = qkps.tile([P, TH], F32, name="q_ps")
                            k_ps = qkps.tile([P, TH], F32, name="k_ps")
                            pslc = slice(pr * P, (pr + 1) * P)
                            for k in range(CK):
                                nc.tensor.matmul(
                                    q_ps[:], wq_sb[:, k, pslc],
                                    xnT[:, k, ts], start=(k == 0),
                                    stop=(k == CK - 1))
                                nc.tensor.matmul(
                                    k_ps[:], wk_sb[:, k, pslc],
                                    xnT[:, k, ts], start=(k == 0),
                                    stop=(k == CK - 1))
                            nc.scalar.activation(qT[:, pr, ts], q_ps[:],
                                                 AF.Copy)
                            nc.scalar.activation(kT[:, pr, ts], k_ps[:],
                                                 AF.Copy)
                        for g in range(th * 4, th * 4 + 4):
                            vp = qkps.tile([P, NHC * HS], F32, name="vp")
                            for k in range(CK):
                                nc.tensor.matmul(
                                    vp[:], xnT[:, k, g * P:(g + 1) * P],
                                    wv_sb[:, k, :], start=(k == 0),
                                    stop=(k == CK - 1))
                            nc.scalar.activation(
                                v_aug[:, g, :, 0:HS],
                                vp[:].rearrange("p (h s) -> p h s", h=NHC),
                                AF.Copy)

                # ---- th0 section (independent of CC_f1 of layer l-1)
                ln_half(xnT, slice(0, TH), xT, ln1g, ln1b, l * CK, 0, lsb)
                qkv_half(0)
                if l == 0:
                    tap("t_xn1", xnT[:, 0, :16])
                    tap("t_q0", qT[:, 0, :16])
                    tap("t_k0", kT[:, 0, :16])
                    tap("t_v0", v_aug[:, 0, 0, :16])
                # ---- consume FFN CC of th1 from previous layer
                if l > 0:
                    consume_cc(cc_pending.pop(0))
                ln_half(xnT, slice(TH, T), xT, ln1g, ln1b, l * CK, 1, lsb)
                qkv_half(1)
                nc.vector.tensor_copy(
                    v_aug[:, :, :, HS:HS + 2],
                    ones_f[:, :1].to_broadcast([P, T // P, NHC, 2]))

                # ---- attention (th-outer so CC_a0 overlaps th1 compute)
                cc_a = []
                with (tc.tile_pool(name="wpps", bufs=2, space="PSUM") as wpps,
                      tc.tile_pool(name="avps", bufs=2, space="PSUM") as avps,
                      tc.tile_pool(name="rbps", bufs=2, space="PSUM") as rbps,
                      tc.tile_pool(name="saps", bufs=2, space="PSUM") as saps):
                    for th in range(2):
                        ts = slice(th * TH, (th + 1) * TH)
                        t0 = th * TH
                        jmax = 4 if th == 0 else 8
                        for pr in range(NPR):
                            rbs = nrm.tile([P, TH], BF16, name="rbs")
                            for h01 in range(2):
                                off = h01 * HS
                                h = 2 * pr + h01
                                o_ps = avps.tile([HS + 2, TH], F32,
                                                 name="o_ps")
                                for j in range(jmax):
                                    ew = ewp.tile([P, TH], BF16, name="ew",
                                                  bufs=2)
                                    wp = wpps.tile([P, TH], F32, name="wp")
                                    nc.tensor.matmul(
                                        wp[:],
                                        kT[off:off + HS, pr,
                                           j * P:(j + 1) * P],
                                        qT[off:off + HS, pr, ts],
                                        start=True, stop=True)
                                    if j * P >= t0:
                                        offc = j * P - t0
                                        if offc > 0:
                                            nc.vector.tensor_scalar_add(
                                                wp[:, :offc], wp[:, :offc],
                                                -1e9)
                                        nc.vector.tensor_add(
                                            wp[:, offc:offc + P],
                                            wp[:, offc:offc + P], m0[:])
                                    nc.scalar.activation(
                                        ew[:], wp[:], AF.Exp, scale=SCALE)
                                    nc.tensor.matmul(
                                        o_ps[:], v_aug[:, j, h, :],
                                        ew[:], start=(j == 0),
                                        stop=(j == jmax - 1))
                                    if l == 0 and h == 0 and j == 0 and th == 0:
                                        tap("t_ew", ew[:, :16])
                                dsb = nrm.tile([P, TH], F32, name="dsb",
                                               bufs=1)
                                nc.scalar.activation(dsb[HS:HS + 1, :],
                                                     o_ps[HS:HS + 1, :],
                                                     AF.Copy)
                                nc.sync.dma_start(dsb[0:1, :],
                                                  dsb[HS:HS + 1, :])
                                nc.vector.reciprocal_approx_fast(
                                    out=dsb[0:1, :], in_=dsb[0:1, :])
                                rr = nrm.tile([P, TH], BF16, name="rr",
                                              bufs=1)
                                nc.vector.tensor_copy(rr[0:1, :], dsb[0:1, :])
                                rb_ps = rbps.tile([P, TH], F32, name="rb_ps")
                                nc.tensor.matmul(
                                    rb_ps[off:off + HS, :],
                                    ones_b[0:1, :HS],
                                    rr[0:1, :], start=True, stop=True)
                                nc.scalar.activation(
                                    rbs[off:off + HS, :],
                                    rb_ps[off:off + HS, :], AF.Copy)
                                if h01 == 0:
                                    nc.scalar.activation(
                                        OT[0:HS, pr, ts], o_ps[0:HS, :],
                                        AF.Copy)
                                else:
                                    otmp = nrm.tile([HS, TH], BF16,
                                                    name="otmp")
                                    nc.scalar.activation(otmp[:],
                                                         o_ps[0:HS, :],
                                                         AF.Copy)
                                    nc.sync.dma_start(OT[HS:P, pr, ts],
                                                      otmp[:])
                            nc.vector.tensor_mul(OT[:, pr, ts], OT[:, pr, ts],
                                                 rbs[:])
                        if l == 0:
                            tap("t_ot", OT[:, 0, :16])
                        # sa for this th right away -> CC overlaps next work
                        sa_all = sap.tile([P, CK, TH], BF16, name="sa_all")
                        for ct in range(CK):
                            sp = saps.tile([P, TH], F32, name="sp")
                            for o in range(NPR):
                                nc.tensor.matmul(
                                    sp[:], php_sb[:, o, ct * P:(ct + 1) * P],
                                    OT[:, o, ts],
                                    start=(o == 0), stop=(o == NPR - 1))
                            nc.vector.tensor_scalar(
                                sa_all[:, ct], sp[:],
                                ab_v[:, l * CK + ct:l * CK + ct + 1],
                                None, ALU.add)
                        if l == 0 and th == 0:
                            tap("t_sa", sa_all[:, 0, :16])
                        cc_a.append(run_cc(sa_all, th))

                # ---- FFN per th (covers the attention CCs)
                cc_f = []
                for th in range(2):
                    ts = slice(th * TH, (th + 1) * TH)
                    consume_cc(cc_a[th])
                    if l == 0 and th == 0:
                        tap("t_x1", xT[:, 0, :16].bitcast(F32))
                    xn2 = xn2p.tile([P, CK, TH], BF16, name="xn2")
                    ln_half(xn2, slice(0, TH), xT, ln2g, ln2b, l * CK, th, lsb)
                    if l == 0 and th == 0:
                        tap("t_xn2", xn2[:, 0, :16])
                    hT = big.tile([P, FCK, TH], BF16, name="hT")
                    with tc.tile_pool(name="hps", bufs=4,
                                      space="PSUM") as hps:
                        for ftg in range(4):
                            w1t = w1p.tile([P, CK, 512], BF16, name="w1t")
                            nc.sync.dma_start(
                                w1t[:],
                                w1_d.ap()[l, ftg]
                                .rearrange("k p f -> p k f"))
                            h_ps = [hps.tile([P, TH], F32, name="h_ps")
                                    for _ in range(4)]
                            for k in range(CK):
                                for fi in range(4):
                                    nc.tensor.matmul(
                                        h_ps[fi][:],
                                        w1t[:, k, fi * P:(fi + 1) * P],
                                        xn2[:, k], start=(k == 0),
                                        stop=(k == CK - 1))
                            for fi in range(4):
                                ft = ftg * 4 + fi
                                nc.scalar.activation(
                                    hT[:, ft], h_ps[fi][:], AF.Gelu,
                                    bias=b1_v[:, l * FCK + ft:
                                              l * FCK + ft + 1])
                    fo_all = sap.tile([P, CK, TH], BF16, name="fo_all")
                    with tc.tile_pool(name="wps", bufs=8,
                                      space="PSUM") as wps:
                        f_ps = [wps.tile([P, TH], F32, name="f_ps")
                                for _ in range(CK)]
                        for fc in range(FCK):
                            w2t = w2p.tile([P, C], BF16, name="w2t")
                            nc.sync.dma_start(w2t[:], w2_d.ap()[l, fc])
                            for ct in range(CK):
                                nc.tensor.matmul(
                                    f_ps[ct][:],
                                    w2t[:, ct * P:(ct + 1) * P],
                                    hT[:, fc], start=(fc == 0),
                                    stop=(fc == FCK - 1))
                        for ct in range(CK):
                            nc.vector.tensor_scalar(
                                fo_all[:, ct], f_ps[ct][:],
                                b2_v[:, l * CK + ct:l * CK + ct + 1],
                                None, ALU.add)
                    if l == 0 and th == 0:
                        tap("t_h", hT[:, 0, :16])
                        tap("t_fo", fo_all[:, 0, :16])
                    cc_f.append(run_cc(fo_all, th))
                # consume FFN th0 CC at layer tail; th1 at next layer top
                consume_cc(cc_f[0])
                cc_pending.append(cc_f[1])
                if l == 0:
                    tap("t_x2", xT[:, 0, :16].bitcast(F32))
                tap(f"t_xl{l}", xT[:, 0, :16].bitcast(F32))

        # ======================= final LN + head =======================
        consume_cc(cc_pending.pop(0))
        xfT = big.tile([P, CK, T], BF16, name="xnT")
        with tc.tile_pool(name="fin", bufs=2) as fin:
            ln_half(xfT, slice(0, TH), xT, lnfg, lnfb, 0, 0, fin)
            ln_half(xfT, slice(TH, T), xT, lnfg, lnfb, 0, 1, fin)
        tap("t_xf", xfT[:, 0, :16])
        with (tc.tile_pool(name="hw", bufs=4) as hwp,
              tc.tile_pool(name="lg", bufs=3) as lgp,
              tc.tile_pool(name="lps", bufs=4, space="PSUM") as lps):
            for vc in range(VC):
                hwc = hwp.tile([P, CK, P], BF16, name="hwc")
                nc.sync.dma_start(
                    hwc[:],
                    hw_d.ap()[:, :, vc * P:(vc + 1) * P]
                    .rearrange("k p v -> p k v"))
                lg = lgp.tile([P, T], BF16, name="lg")
                for th in range(2):
                    ts = slice(th * TH, (th + 1) * TH)
                    lp = lps.tile([P, TH], F32, name="lp")
                    for k in range(CK):
                        nc.tensor.matmul(lp[:], hwc[:, k, :], xfT[:, k, ts],
                                         start=(k == 0), stop=(k == CK - 1))
                    nc.vector.tensor_scalar(
                        lg[:, ts], lp[:], hb_v[:, vc:vc + 1], None, ALU.add)
                if vc == 0:
                    tap("t_lg", lg[:, :16])
                nc.sync.dma_start(logits_d.ap()[vc], lg[:])

    nc.compile()
    return nc


# ----------------------------------------------------------------------------
# host entry
# ----------------------------------------------------------------------------

def kernel(**inputs):
    from concourse.bass_utils import run_bass_kernel_spmd

    if "nc" not in _CACHE:
        _CACHE["nc"] = build_nc()
    nc = _CACHE["nc"]

    if "in_maps" not in _CACHE:
        _CACHE["in_maps"] = [_prep_core(inputs, c) for c in range(8)]
    in_maps = _CACHE["in_maps"]
    res = run_bass_kernel_spmd(nc, in_maps, core_ids=list(range(8)))
    out = np.zeros((B, T, V), np.float32)
    for c in range(8):
        b, tp = c // 2, c % 2
        lg = np.asarray(res.results[c]["logits"], np.float32)  # [VC, P, T]
        out[b, :, tp * VS:(tp + 1) * VS] = lg.transpose(2, 0, 1).reshape(T, VS)
    return out
